# revision 10
# baseline (speedup 1.0000x reference)
"""Int8Linear Trainium2 kernel v2 (8 NeuronCores, batch-sharded).

Math (matches the jax reference):
  x_q   = round(x / s) + zp  (per-tensor affine, int8 range, no wrap)
  xc    = x_q - zp = round(x / s)              <- zp cancels
  wc    = w_q - w_zp                           <- host-prepped, exact in bf16
  out   = (xc @ wc.T) * (s * w_scale) + bias

Changes vs the first working version:
  - x loaded ONCE, kept SBUF-resident; quantize runs in-place (no 2nd HBM
    pass over x).
  - wc = w_q - w_zp precomputed on host (exact: integer values <= 256 in
    bf16), so no on-device weight prep pass.
  - bias folded into the GEMM as a K=1 matmul row (rhs = bias/(s*w_scale)
    in bf16, lhsT = ones) -> no per-tile DVE bias add.
  - one partition_all_reduce on stacked [max,-min] [128,2]; the 8-core
    AllReduce then runs on the full [128,2] buffer (same latency floor as
    [1,2]), so the global result lands on every partition and the scale
    chain is computed replicated -- no partition_broadcast at all.
  - bias arrives bf16, w_scale arrives host-replicated [128,1]; no mm
    output.

Device strategy per core c (core c owns batch c, M=2048 tokens):
  pass 1: per-k-tile max/min reduce -> partition_all_reduce(max) on
  [max,-min] -> 8-core AllReduce(max) on [128,2] -> replicated scale
  chain (s, 1/s, ss = s*w_scale, bias_q = bias/ss).
  quantize: q = rne(x * 1/s) via the +/-1.5*2^23 magic trick, in-place
  f32 op then bf16 cast (exact: integers <= 256).
  GEMM: 16 m-tiles x 2 halves; per half: 8 k x 4 o-chunk matmuls (N=512)
  + 4 bias matmuls (K=1); PSUM [128,2048] x 2 ping-pong.
  epilogue: ACT copy psum*ss -> SBUF f32 -> DMA out.
"""

import numpy as np
import ml_dtypes

B, S, IN, OUT = 8, 2048, 1024, 4096
NCORES = 8
KT = IN // 128          # 8 k-tiles
MT = S // 128           # 16 m-tiles
HALF = OUT // 2         # 2048
MAGIC = 12582912.0      # 1.5 * 2**23: forces f32 RNE at integer granularity

_cache = {}


def _build_program_v2(reps=1, skip_cc=False, skip_gemm=False):
    # skip_cc / skip_gemm are timing-decomposition knobs (never used by
    # kernel()): skip_cc uses the core-local max/min instead of the 8-core
    # AllReduce; skip_gemm ends the program after quantize.
    import concourse.bass as bass
    import concourse.mybir as mybir
    import concourse.bacc as bacc
    import concourse.tile as tile
    from concourse import bass_isa

    f32 = mybir.dt.float32
    bf16 = mybir.dt.bfloat16
    Alu = mybir.AluOpType
    Copy = mybir.ActivationFunctionType.Copy

    nc = bacc.Bacc(
        "TRN2",
        target_bir_lowering=False,
        debug=False,
        enable_asserts=True,
        num_devices=NCORES,
    )

    xt_d = nc.dram_tensor("xt", [IN, S], f32, kind="ExternalInput").ap()
    wc_d = nc.dram_tensor("wc", [IN, OUT], bf16, kind="ExternalInput").ap()
    bias_d = nc.dram_tensor("biasb", [1, OUT], bf16, kind="ExternalInput").ap()
    wsc_d = nc.dram_tensor("wsc", [128, 1], f32, kind="ExternalInput").ap()
    out_d = nc.dram_tensor("out", [S, OUT], f32, kind="ExternalOutput").ap()

    with tile.TileContext(nc) as tc:
        with (
            tc.tile_pool(name="xin", bufs=KT) as xin_pool,
            tc.tile_pool(name="xq", bufs=KT) as xq_pool,
            tc.tile_pool(name="wq", bufs=KT) as wq_pool,
            tc.tile_pool(name="stats", bufs=1) as stats,
            tc.tile_pool(name="osb", bufs=3) as osb_pool,
            tc.tile_pool(name="psum", bufs=2, space="PSUM") as psum_pool,
            tc.tile_pool(name="dram", bufs=1, space="DRAM") as dram_pool,
        ):
          for _rep in range(reps):
            # ---- aux inputs / constants ----
            bias_sb = stats.tile([1, OUT], bf16, tag="bias_sb")
            nc.sync.dma_start(bias_sb[:], bias_d[:])
            wsc_sb = stats.tile([128, 1], f32, tag="wsc_sb")
            nc.sync.dma_start(wsc_sb[:], wsc_d[:])
            ones = stats.tile([1, 128], bf16, tag="ones")
            nc.vector.memset(ones[:], 1.0)

            # ---- pass 1: load x, per-tile max/min ----
            mm8 = stats.tile([128, 2 * KT], f32, tag="mm8")  # maxes | -mins
            xin = []
            for k in range(KT):
                x_t = xin_pool.tile([128, S], f32, tag="xin")
                nc.sync.dma_start(x_t[:], xt_d[k * 128:(k + 1) * 128, :])
                nc.vector.tensor_reduce(
                    mm8[:, k:k + 1], x_t[:], axis=mybir.AxisListType.X, op=Alu.max)
                nc.vector.tensor_reduce(
                    mm8[:, KT + k:KT + k + 1], x_t[:], axis=mybir.AxisListType.X,
                    op=Alu.min)
                xin.append(x_t)
            mm2 = stats.tile([128, 2], f32, tag="mm2")  # [max, min] per part
            nc.vector.tensor_reduce(
                mm2[:, 0:1], mm8[:, 0:KT], axis=mybir.AxisListType.X, op=Alu.max)
            nc.vector.tensor_reduce(
                mm2[:, 1:2], mm8[:, KT:2 * KT], axis=mybir.AxisListType.X,
                op=Alu.min)
            # negate min -> [max, -min], one cross-partition all-reduce(max)
            nc.vector.tensor_scalar_mul(mm2[:, 1:2], mm2[:, 1:2], -1.0)
            gmm = stats.tile([128, 2], f32, tag="gmm")
            nc.gpsimd.partition_all_reduce(gmm[:], mm2[:], channels=128,
                                           reduce_op=bass_isa.ReduceOp.max)

            # ---- 8-core AllReduce(max) on [128, 2] ----
            # partition_all_reduce already left the shard [max,-min] on every
            # partition, so running the collective on the full [128,2] buffer
            # (same ~10us latency floor as [1,2]) hands the global result back
            # to every partition -- no partition_broadcast needed afterwards.
            if skip_cc:
                cc_res = gmm
            else:
                cc_in = dram_pool.tile([128, 2], f32, tag="cc_in")
                cc_out = dram_pool.tile([128, 2], f32, tag="cc_out")
                nc.gpsimd.dma_start(cc_in[:], gmm[:])
                nc.gpsimd.collective_compute(
                    "AllReduce",
                    Alu.max,
                    replica_groups=[list(range(NCORES))],
                    ins=[cc_in.opt()],
                    outs=[cc_out.opt()],
                )
                cc_res = stats.tile([128, 2], f32, tag="cc_res")
                nc.gpsimd.dma_start(cc_res[:], cc_out[:])

            # ---- chain (replicated on all 128 partitions): s, 1/s, ss ----
            d128 = stats.tile([128, 1], f32, tag="d128")
            nc.vector.tensor_tensor(d128[:], cc_res[:, 0:1], cc_res[:, 1:2],
                                    op=Alu.add)
            scale128 = stats.tile([128, 1], f32, tag="scale128")
            nc.vector.tensor_scalar_mul(scale128[:], d128[:], 1.0 / 255.0)
            inv128_t = stats.tile([128, 1], f32, tag="inv128")
            nc.vector.reciprocal(inv128_t[:], scale128[:])
            ss128_t = stats.tile([128, 1], f32, tag="ss128")
            nc.vector.tensor_tensor(ss128_t[:], scale128[:], wsc_sb[:],
                                    op=Alu.mult)
            issm = stats.tile([1, 1], f32, tag="issm")
            nc.vector.reciprocal(issm[:], ss128_t[0:1, :])
            bias_q = stats.tile([1, OUT], bf16, tag="bias_q")
            nc.vector.tensor_scalar(bias_q[:], bias_sb[:], issm[:], None,
                                    op0=Alu.mult)
            inv128 = inv128_t[:]
            ss128 = ss128_t[:]

            # ---- weights ----
            wq = []
            for k in range(KT):
                w_t = wq_pool.tile([128, OUT], bf16, tag="wq")
                nc.sync.dma_start(w_t[:], wc_d[k * 128:(k + 1) * 128, :])
                wq.append(w_t)

            # ---- quantize x in place -> bf16 integers ----
            xq = []
            for k in range(KT):
                x_t = xin[k]
                nc.vector.tensor_scalar(x_t[:], x_t[:], inv128[:], MAGIC,
                                        op0=Alu.mult, op1=Alu.add)
                q_t = xq_pool.tile([128, S], bf16, tag="xq")
                nc.vector.tensor_scalar(q_t[:], x_t[:], MAGIC, None,
                                        op0=Alu.subtract)
                xq.append(q_t)

            # ---- GEMM (+ bias fold) + epilogue ----
            for m in range(MT if not skip_gemm else 0):
                for h in range(2):
                    ps = psum_pool.tile([128, HALF], f32, tag="ps")
                    for k in range(KT):
                        lhsT = xq[k][:, m * 128:(m + 1) * 128]
                        for o in range(4):
                            col = h * HALF + o * 512
                            nc.tensor.matmul(
                                ps[:, o * 512:(o + 1) * 512],
                                lhsT,
                                wq[k][:, col:col + 512],
                                start=(k == 0),
                                stop=False,
                            )
                    for o in range(4):
                        col = h * HALF + o * 512
                        nc.tensor.matmul(
                            ps[:, o * 512:(o + 1) * 512],
                            ones[:],
                            bias_q[0:1, col:col + 512],
                            start=False,
                            stop=True,
                        )
                    o_t = osb_pool.tile([128, HALF], f32, tag="osb")
                    nc.scalar.activation(o_t[:], ps[:], Copy, bias=0.0,
                                         scale=ss128)
                    nc.sync.dma_start(
                        out_d[m * 128:(m + 1) * 128, h * HALF:(h + 1) * HALF],
                        o_t[:])

    nc.compile()
    return nc


def _prep_host(x, w_q, w_scale, w_zp, bias):
    x = np.asarray(x, dtype=np.float32)
    w_q = np.asarray(w_q)
    # wc = w_q - w_zp: integer-valued, |.| <= 256 -> exact in bf16
    wc = np.ascontiguousarray(
        (w_q.T.astype(np.float32) - np.float32(w_zp))).astype(ml_dtypes.bfloat16)
    biasb = np.asarray(bias, dtype=np.float32).reshape(1, OUT).astype(
        ml_dtypes.bfloat16)
    wsc = np.full((128, 1), np.float32(w_scale), dtype=np.float32)
    xts = [np.ascontiguousarray(x[c].T) for c in range(NCORES)]
    in_maps = [{"xt": xts[c], "wc": wc, "biasb": biasb, "wsc": wsc}
               for c in range(NCORES)]
    return in_maps


def kernel(x, w_q, w_scale, w_zp, bias, _bench=False):
    from concourse.bass_utils import run_bass_kernel_spmd

    in_maps = _prep_host(x, w_q, w_scale, w_zp, bias)
    if "nc" not in _cache:
        _cache["nc"] = _build_program_v2()
    res = run_bass_kernel_spmd(_cache["nc"], in_maps, list(range(NCORES)))
    out = np.stack([res.results[c]["out"] for c in range(NCORES)], axis=0)
    if _bench:
        return out, res
    return out


# revision 16
# speedup vs baseline: 1.2814x; 1.2814x over previous
"""Int8Linear Trainium2 kernel v2 (8 NeuronCores, batch-sharded).

Math (matches the jax reference):
  x_q   = round(x / s) + zp  (per-tensor affine, int8 range, no wrap)
  xc    = x_q - zp = round(x / s)              <- zp cancels
  wc    = w_q - w_zp                           <- host-prepped, exact in bf16
  out   = (xc @ wc.T) * (s * w_scale) + bias

Changes vs the first working version:
  - x loaded ONCE, kept SBUF-resident; quantize runs in-place (no 2nd HBM
    pass over x).
  - wc = w_q - w_zp precomputed on host (exact: integer values <= 256 in
    bf16), so no on-device weight prep pass.
  - bias folded into the GEMM as a K=1 matmul row (rhs = bias/(s*w_scale)
    in bf16, lhsT = ones) -> no per-tile DVE bias add.
  - one partition_all_reduce on stacked [max,-min] [128,2]; the 8-core
    AllReduce then runs on the full [128,2] buffer (same latency floor as
    [1,2]), so the global result lands on every partition and the scale
    chain is computed replicated -- no partition_broadcast at all.
  - bias arrives bf16, w_scale arrives host-replicated [128,1]; no mm
    output.

Device strategy per core c (core c owns batch c, M=2048 tokens):
  pass 1: per-k-tile max/min reduce -> partition_all_reduce(max) on
  [max,-min] -> 8-core AllReduce(max) on [128,2] -> replicated scale
  chain (s, 1/s, ss = s*w_scale, bias_q = bias/ss).
  quantize: q = rne(x * 1/s) via the +/-1.5*2^23 magic trick, in-place
  f32 op then bf16 cast (exact: integers <= 256).
  GEMM: 16 m-tiles x 2 halves; per half: 8 k x 4 o-chunk matmuls (N=512)
  + 4 bias matmuls (K=1); PSUM [128,2048] x 2 ping-pong.
  epilogue: ACT copy psum*ss -> SBUF f32 -> DMA out.
"""

import numpy as np
import ml_dtypes

B, S, IN, OUT = 8, 2048, 1024, 4096
NCORES = 8
KT = IN // 128          # 8 k-tiles
MT = S // 128           # 16 m-tiles
HALF = OUT // 2         # 2048
MAGIC = 12582912.0      # 1.5 * 2**23: forces f32 RNE at integer granularity

_cache = {}


def _build_program_v2(reps=1, skip_cc=False, skip_gemm=False):
    # skip_cc / skip_gemm are timing-decomposition knobs (never used by
    # kernel()): skip_cc uses the core-local max/min instead of the 8-core
    # AllReduce; skip_gemm ends the program after quantize.
    import concourse.bass as bass
    import concourse.mybir as mybir
    import concourse.bacc as bacc
    import concourse.tile as tile
    from concourse import bass_isa

    f32 = mybir.dt.float32
    bf16 = mybir.dt.bfloat16
    Alu = mybir.AluOpType
    Copy = mybir.ActivationFunctionType.Copy

    nc = bacc.Bacc(
        "TRN2",
        target_bir_lowering=False,
        debug=False,
        enable_asserts=True,
        num_devices=NCORES,
    )

    xt_d = nc.dram_tensor("xt", [IN, S], f32, kind="ExternalInput").ap()
    wc_d = nc.dram_tensor("wc", [IN, OUT], bf16, kind="ExternalInput").ap()
    bias_d = nc.dram_tensor("biasb", [1, OUT], bf16, kind="ExternalInput").ap()
    wsc_d = nc.dram_tensor("wsc", [128, 1], f32, kind="ExternalInput").ap()
    out_d = nc.dram_tensor("out", [S, OUT], f32, kind="ExternalOutput").ap()

    with tile.TileContext(nc) as tc:
        with (
            tc.tile_pool(name="xin", bufs=KT) as xin_pool,
            tc.tile_pool(name="xq", bufs=KT) as xq_pool,
            tc.tile_pool(name="wq", bufs=KT) as wq_pool,
            tc.tile_pool(name="stats", bufs=1) as stats,
            tc.tile_pool(name="osb", bufs=3) as osb_pool,
            tc.tile_pool(name="psum", bufs=2, space="PSUM") as psum_pool,
            tc.tile_pool(name="dram", bufs=1, space="DRAM") as dram_pool,
        ):
          for _rep in range(reps):
            # ---- aux inputs / constants ----
            bias_sb = stats.tile([1, OUT], bf16, tag="bias_sb")
            nc.sync.dma_start(bias_sb[:], bias_d[:])
            wsc_sb = stats.tile([128, 1], f32, tag="wsc_sb")
            nc.sync.dma_start(wsc_sb[:], wsc_d[:])
            ones = stats.tile([1, 128], bf16, tag="ones")
            nc.vector.memset(ones[:], 1.0)

            # ---- pass 1: load x, per-tile max/min ----
            mm8 = stats.tile([128, 2 * KT], f32, tag="mm8")  # maxes | -mins
            xin = []
            for k in range(KT):
                x_t = xin_pool.tile([128, S], f32, tag="xin")
                nc.sync.dma_start(x_t[:], xt_d[k * 128:(k + 1) * 128, :])
                nc.vector.tensor_reduce(
                    mm8[:, k:k + 1], x_t[:], axis=mybir.AxisListType.X, op=Alu.max)
                nc.vector.tensor_reduce(
                    mm8[:, KT + k:KT + k + 1], x_t[:], axis=mybir.AxisListType.X,
                    op=Alu.min)
                xin.append(x_t)
            mm2 = stats.tile([128, 2], f32, tag="mm2")  # [max, min] per part
            nc.vector.tensor_reduce(
                mm2[:, 0:1], mm8[:, 0:KT], axis=mybir.AxisListType.X, op=Alu.max)
            nc.vector.tensor_reduce(
                mm2[:, 1:2], mm8[:, KT:2 * KT], axis=mybir.AxisListType.X,
                op=Alu.min)
            # negate min -> [max, -min], one cross-partition all-reduce(max)
            nc.vector.tensor_scalar_mul(mm2[:, 1:2], mm2[:, 1:2], -1.0)
            gmm = stats.tile([128, 2], f32, tag="gmm")
            nc.gpsimd.partition_all_reduce(gmm[:], mm2[:], channels=128,
                                           reduce_op=bass_isa.ReduceOp.max)

            # ---- 8-core AllReduce(max) on [128, 2] ----
            # partition_all_reduce already left the shard [max,-min] on every
            # partition, so running the collective on the full [128,2] buffer
            # (same ~10us latency floor as [1,2]) hands the global result back
            # to every partition -- no partition_broadcast needed afterwards.
            if skip_cc:
                cc_res = gmm
            else:
                cc_in = dram_pool.tile([128, 2], f32, tag="cc_in")
                cc_out = dram_pool.tile([128, 2], f32, tag="cc_out")
                nc.gpsimd.dma_start(cc_in[:], gmm[:])
                nc.gpsimd.collective_compute(
                    "AllReduce",
                    Alu.max,
                    replica_groups=[list(range(NCORES))],
                    ins=[cc_in.opt()],
                    outs=[cc_out.opt()],
                )
                cc_res = stats.tile([128, 2], f32, tag="cc_res")
                nc.gpsimd.dma_start(cc_res[:], cc_out[:])

            # ---- chain (replicated on all 128 partitions): s, 1/s, ss ----
            d128 = stats.tile([128, 1], f32, tag="d128")
            nc.vector.tensor_tensor(d128[:], cc_res[:, 0:1], cc_res[:, 1:2],
                                    op=Alu.add)
            scale128 = stats.tile([128, 1], f32, tag="scale128")
            nc.vector.tensor_scalar_mul(scale128[:], d128[:], 1.0 / 255.0)
            inv128_t = stats.tile([128, 1], f32, tag="inv128")
            nc.vector.reciprocal(inv128_t[:], scale128[:])
            ss128_t = stats.tile([128, 1], f32, tag="ss128")
            nc.vector.tensor_tensor(ss128_t[:], scale128[:], wsc_sb[:],
                                    op=Alu.mult)
            issm = stats.tile([1, 1], f32, tag="issm")
            nc.vector.reciprocal(issm[:], ss128_t[0:1, :])
            bias_q = stats.tile([1, OUT], bf16, tag="bias_q")
            nc.vector.tensor_scalar(bias_q[:], bias_sb[:], issm[:], None,
                                    op0=Alu.mult)
            inv128 = inv128_t[:]
            ss128 = ss128_t[:]

            # ---- weights ----
            wq = []
            for k in range(KT):
                w_t = wq_pool.tile([128, OUT], bf16, tag="wq")
                nc.sync.dma_start(w_t[:], wc_d[k * 128:(k + 1) * 128, :])
                wq.append(w_t)

            # ---- quantize x in place -> bf16 integers ----
            xq = []
            for k in range(KT):
                x_t = xin[k]
                nc.vector.tensor_scalar(x_t[:], x_t[:], inv128[:], MAGIC,
                                        op0=Alu.mult, op1=Alu.add)
                q_t = xq_pool.tile([128, S], bf16, tag="xq")
                nc.vector.tensor_scalar(q_t[:], x_t[:], MAGIC, None,
                                        op0=Alu.subtract)
                xq.append(q_t)

            # ---- GEMM (+ bias fold) + epilogue ----
            for m in range(MT if not skip_gemm else 0):
                for h in range(2):
                    ps = psum_pool.tile([128, HALF], f32, tag="ps")
                    for k in range(KT):
                        lhsT = xq[k][:, m * 128:(m + 1) * 128]
                        for o in range(4):
                            col = h * HALF + o * 512
                            nc.tensor.matmul(
                                ps[:, o * 512:(o + 1) * 512],
                                lhsT,
                                wq[k][:, col:col + 512],
                                start=(k == 0),
                                stop=False,
                            )
                    for o in range(4):
                        col = h * HALF + o * 512
                        nc.tensor.matmul(
                            ps[:, o * 512:(o + 1) * 512],
                            ones[:],
                            bias_q[0:1, col:col + 512],
                            start=False,
                            stop=True,
                        )
                    o_t = osb_pool.tile([128, HALF], f32, tag="osb")
                    nc.scalar.activation(o_t[:], ps[:], Copy, bias=0.0,
                                         scale=ss128)
                    nc.sync.dma_start(
                        out_d[m * 128:(m + 1) * 128, h * HALF:(h + 1) * HALF],
                        o_t[:])

    nc.compile()
    return nc


def _prep_host(x, w_q, w_scale, w_zp, bias):
    x = np.asarray(x, dtype=np.float32)
    w_q = np.asarray(w_q)
    # wc = w_q - w_zp: integer-valued, |.| <= 256 -> exact in bf16
    wc = np.ascontiguousarray(
        (w_q.T.astype(np.float32) - np.float32(w_zp))).astype(ml_dtypes.bfloat16)
    biasb = np.asarray(bias, dtype=np.float32).reshape(1, OUT).astype(
        ml_dtypes.bfloat16)
    wsc = np.full((128, 1), np.float32(w_scale), dtype=np.float32)
    xts = [np.ascontiguousarray(x[c].T) for c in range(NCORES)]
    in_maps = [{"xt": xts[c], "wc": wc, "biasb": biasb, "wsc": wsc}
               for c in range(NCORES)]
    return in_maps


def kernel(x, w_q, w_scale, w_zp, bias, _bench=False):
    from concourse.bass_utils import run_bass_kernel_spmd

    in_maps = _prep_host(x, w_q, w_scale, w_zp, bias)
    if "nc" not in _cache:
        _cache["nc"] = _build_program_v2()
    res = run_bass_kernel_spmd(_cache["nc"], in_maps, list(range(NCORES)))
    out = np.stack([res.results[c]["out"] for c in range(NCORES)], axis=0)
    if _bench:
        return out, res
    return out


# revision 17
# speedup vs baseline: 1.5047x; 1.1742x over previous
"""Int8Linear Trainium2 kernel v2 (8 NeuronCores, batch-sharded).

Math (matches the jax reference):
  x_q   = round(x / s) + zp  (per-tensor affine, int8 range, no wrap)
  xc    = x_q - zp = round(x / s)              <- zp cancels
  wc    = w_q - w_zp                           <- host-prepped, exact in bf16
  out   = (xc @ wc.T) * (s * w_scale) + bias

Changes vs the first working version:
  - x loaded ONCE, kept SBUF-resident; quantize runs in-place (no 2nd HBM
    pass over x).
  - wc = w_q - w_zp precomputed on host (exact: integer values <= 256 in
    bf16), so no on-device weight prep pass.
  - bias folded into the GEMM as a K=1 matmul row (rhs = bias/(s*w_scale)
    in bf16, lhsT = ones) -> no per-tile DVE bias add.
  - one partition_all_reduce on stacked [max,-min] [128,2]; the 8-core
    AllReduce then runs on the full [128,2] buffer (same latency floor as
    [1,2]), so the global result lands on every partition and the scale
    chain is computed replicated -- no partition_broadcast at all.
  - bias arrives bf16, w_scale arrives host-replicated [128,1]; no mm
    output.

Device strategy per core c (core c owns batch c, M=2048 tokens):
  pass 1: per-k-tile max/min reduce -> partition_all_reduce(max) on
  [max,-min] -> 8-core AllReduce(max) on [128,2] -> replicated scale
  chain (s, 1/s, ss = s*w_scale, bias_q = bias/ss).
  quantize: q = rne(x * 1/s) via the +/-1.5*2^23 magic trick, in-place
  f32 op then bf16 cast (exact: integers <= 256).
  GEMM: 16 m-tiles x 2 halves; per half: 8 k x 4 o-chunk matmuls (N=512)
  + 4 bias matmuls (K=1); PSUM [128,2048] x 2 ping-pong.
  epilogue: ACT copy psum*ss -> SBUF f32 -> DMA out.
"""

import numpy as np
import ml_dtypes

B, S, IN, OUT = 8, 2048, 1024, 4096
NCORES = 8
KT = IN // 128          # 8 k-tiles
MT = S // 128           # 16 m-tiles
HALF = OUT // 2         # 2048
MAGIC = 12582912.0      # 1.5 * 2**23: forces f32 RNE at integer granularity

_cache = {}


def _build_program_v2(reps=1, skip_cc=False, skip_gemm=False):
    # skip_cc / skip_gemm are timing-decomposition knobs (never used by
    # kernel()): skip_cc uses the core-local max/min instead of the 8-core
    # AllReduce; skip_gemm ends the program after quantize.
    import concourse.bass as bass
    import concourse.mybir as mybir
    import concourse.bacc as bacc
    import concourse.tile as tile
    from concourse import bass_isa

    f32 = mybir.dt.float32
    bf16 = mybir.dt.bfloat16
    Alu = mybir.AluOpType
    Copy = mybir.ActivationFunctionType.Copy

    nc = bacc.Bacc(
        "TRN2",
        target_bir_lowering=False,
        debug=False,
        enable_asserts=True,
        num_devices=NCORES,
    )

    xt_d = nc.dram_tensor("xt", [IN, S], f32, kind="ExternalInput").ap()
    wc_d = nc.dram_tensor("wc", [IN, OUT], bf16, kind="ExternalInput").ap()
    bias_d = nc.dram_tensor("biasb", [1, OUT], bf16, kind="ExternalInput").ap()
    wsc_d = nc.dram_tensor("wsc", [128, 1], f32, kind="ExternalInput").ap()
    # Output in bf16: adds <=2^-9 relative rounding on outputs (total error
    # ~3e-3 vs the 2e-2 gate) and halves the per-call output buffer, which
    # is what bounds safe dispatch-pipeline depth; kernel() upcasts to f32.
    out_d = nc.dram_tensor("out", [S, OUT], bf16, kind="ExternalOutput").ap()

    with tile.TileContext(nc) as tc:
        with (
            tc.tile_pool(name="xin", bufs=KT) as xin_pool,
            tc.tile_pool(name="xq", bufs=KT) as xq_pool,
            tc.tile_pool(name="wq", bufs=KT) as wq_pool,
            tc.tile_pool(name="stats", bufs=1) as stats,
            tc.tile_pool(name="osb", bufs=3) as osb_pool,
            tc.tile_pool(name="psum", bufs=2, space="PSUM") as psum_pool,
            tc.tile_pool(name="dram", bufs=1, space="DRAM") as dram_pool,
        ):
          for _rep in range(reps):
            # ---- aux inputs / constants ----
            bias_sb = stats.tile([1, OUT], bf16, tag="bias_sb")
            nc.sync.dma_start(bias_sb[:], bias_d[:])
            wsc_sb = stats.tile([128, 1], f32, tag="wsc_sb")
            nc.sync.dma_start(wsc_sb[:], wsc_d[:])
            ones = stats.tile([1, 128], bf16, tag="ones")
            nc.vector.memset(ones[:], 1.0)

            # ---- pass 1: load x, per-tile max/min ----
            mm8 = stats.tile([128, 2 * KT], f32, tag="mm8")  # maxes | -mins
            xin = []
            for k in range(KT):
                x_t = xin_pool.tile([128, S], f32, tag="xin")
                nc.sync.dma_start(x_t[:], xt_d[k * 128:(k + 1) * 128, :])
                nc.vector.tensor_reduce(
                    mm8[:, k:k + 1], x_t[:], axis=mybir.AxisListType.X, op=Alu.max)
                nc.vector.tensor_reduce(
                    mm8[:, KT + k:KT + k + 1], x_t[:], axis=mybir.AxisListType.X,
                    op=Alu.min)
                xin.append(x_t)
            mm2 = stats.tile([128, 2], f32, tag="mm2")  # [max, min] per part
            nc.vector.tensor_reduce(
                mm2[:, 0:1], mm8[:, 0:KT], axis=mybir.AxisListType.X, op=Alu.max)
            nc.vector.tensor_reduce(
                mm2[:, 1:2], mm8[:, KT:2 * KT], axis=mybir.AxisListType.X,
                op=Alu.min)
            # negate min -> [max, -min], one cross-partition all-reduce(max)
            nc.vector.tensor_scalar_mul(mm2[:, 1:2], mm2[:, 1:2], -1.0)
            gmm = stats.tile([128, 2], f32, tag="gmm")
            nc.gpsimd.partition_all_reduce(gmm[:], mm2[:], channels=128,
                                           reduce_op=bass_isa.ReduceOp.max)

            # ---- 8-core AllReduce(max) on [128, 2] ----
            # partition_all_reduce already left the shard [max,-min] on every
            # partition, so running the collective on the full [128,2] buffer
            # (same ~10us latency floor as [1,2]) hands the global result back
            # to every partition -- no partition_broadcast needed afterwards.
            if skip_cc:
                cc_res = gmm
            else:
                cc_in = dram_pool.tile([128, 2], f32, tag="cc_in")
                cc_out = dram_pool.tile([128, 2], f32, tag="cc_out")
                nc.gpsimd.dma_start(cc_in[:], gmm[:])
                nc.gpsimd.collective_compute(
                    "AllReduce",
                    Alu.max,
                    replica_groups=[list(range(NCORES))],
                    ins=[cc_in.opt()],
                    outs=[cc_out.opt()],
                )
                cc_res = stats.tile([128, 2], f32, tag="cc_res")
                nc.gpsimd.dma_start(cc_res[:], cc_out[:])

            # ---- chain (replicated on all 128 partitions): s, 1/s, ss ----
            d128 = stats.tile([128, 1], f32, tag="d128")
            nc.vector.tensor_tensor(d128[:], cc_res[:, 0:1], cc_res[:, 1:2],
                                    op=Alu.add)
            scale128 = stats.tile([128, 1], f32, tag="scale128")
            nc.vector.tensor_scalar_mul(scale128[:], d128[:], 1.0 / 255.0)
            inv128_t = stats.tile([128, 1], f32, tag="inv128")
            nc.vector.reciprocal(inv128_t[:], scale128[:])
            ss128_t = stats.tile([128, 1], f32, tag="ss128")
            nc.vector.tensor_tensor(ss128_t[:], scale128[:], wsc_sb[:],
                                    op=Alu.mult)
            issm = stats.tile([1, 1], f32, tag="issm")
            nc.vector.reciprocal(issm[:], ss128_t[0:1, :])
            bias_q = stats.tile([1, OUT], bf16, tag="bias_q")
            nc.vector.tensor_scalar(bias_q[:], bias_sb[:], issm[:], None,
                                    op0=Alu.mult)
            inv128 = inv128_t[:]
            ss128 = ss128_t[:]

            # ---- weights ----
            wq = []
            for k in range(KT):
                w_t = wq_pool.tile([128, OUT], bf16, tag="wq")
                nc.sync.dma_start(w_t[:], wc_d[k * 128:(k + 1) * 128, :])
                wq.append(w_t)

            # ---- quantize x in place -> bf16 integers ----
            xq = []
            for k in range(KT):
                x_t = xin[k]
                nc.vector.tensor_scalar(x_t[:], x_t[:], inv128[:], MAGIC,
                                        op0=Alu.mult, op1=Alu.add)
                q_t = xq_pool.tile([128, S], bf16, tag="xq")
                nc.vector.tensor_scalar(q_t[:], x_t[:], MAGIC, None,
                                        op0=Alu.subtract)
                xq.append(q_t)

            # ---- GEMM (+ bias fold) + epilogue ----
            for m in range(MT if not skip_gemm else 0):
                for h in range(2):
                    ps = psum_pool.tile([128, HALF], f32, tag="ps")
                    for k in range(KT):
                        lhsT = xq[k][:, m * 128:(m + 1) * 128]
                        for o in range(4):
                            col = h * HALF + o * 512
                            nc.tensor.matmul(
                                ps[:, o * 512:(o + 1) * 512],
                                lhsT,
                                wq[k][:, col:col + 512],
                                start=(k == 0),
                                stop=False,
                            )
                    for o in range(4):
                        col = h * HALF + o * 512
                        nc.tensor.matmul(
                            ps[:, o * 512:(o + 1) * 512],
                            ones[:],
                            bias_q[0:1, col:col + 512],
                            start=False,
                            stop=True,
                        )
                    o_t = osb_pool.tile([128, HALF], bf16, tag="osb")
                    nc.scalar.activation(o_t[:], ps[:], Copy, bias=0.0,
                                         scale=ss128)
                    nc.sync.dma_start(
                        out_d[m * 128:(m + 1) * 128, h * HALF:(h + 1) * HALF],
                        o_t[:])

    nc.compile()
    return nc


def _prep_host(x, w_q, w_scale, w_zp, bias):
    x = np.asarray(x, dtype=np.float32)
    w_q = np.asarray(w_q)
    # wc = w_q - w_zp: integer-valued, |.| <= 256 -> exact in bf16
    wc = np.ascontiguousarray(
        (w_q.T.astype(np.float32) - np.float32(w_zp))).astype(ml_dtypes.bfloat16)
    biasb = np.asarray(bias, dtype=np.float32).reshape(1, OUT).astype(
        ml_dtypes.bfloat16)
    wsc = np.full((128, 1), np.float32(w_scale), dtype=np.float32)
    xts = [np.ascontiguousarray(x[c].T) for c in range(NCORES)]
    in_maps = [{"xt": xts[c], "wc": wc, "biasb": biasb, "wsc": wsc}
               for c in range(NCORES)]
    return in_maps


def kernel(x, w_q, w_scale, w_zp, bias, _bench=False):
    from concourse.bass_utils import run_bass_kernel_spmd

    in_maps = _prep_host(x, w_q, w_scale, w_zp, bias)
    if "nc" not in _cache:
        _cache["nc"] = _build_program_v2()
    res = run_bass_kernel_spmd(_cache["nc"], in_maps, list(range(NCORES)))
    out = np.stack([res.results[c]["out"].astype(np.float32)
                    for c in range(NCORES)], axis=0)
    if _bench:
        return out, res
    return out
